# revision 10
# baseline (speedup 1.0000x reference)
"""Trainium2 Bass kernel for nn_CGWeight: weighted Clebsch-Gordan tensor product.

out[k] = nan_to_num( sum_c w_c * sum_{i,j} CG_c[i,j,k] * A[ai_c][i] * H[hi_c][j] )

One small fp32 matmul against a precomputed [75, 450] constant G:
  lhsT [75, 65]  = host-packed block-diagonal: concat(A) rows 0-24 col 0,
                   concat(H) rows 25-49 col 32, weight rows 50-74 col 64
                   (cols 0/32/64 so result rows hit legal DVE start partitions)
  rhs  [75, 450] = G: rows 0-24 CG contraction (cols ordered (k, c, j)),
                   rows 25-49 H-scatter 0/1, rows 50-74 w-scatter 0/1
  PSUM [65, 450]: row0 = B[(k,c,j)] = sum_i CG_c[i,j,k] A_i,
                  row32 = H[hi_c][j] (k-periodic), row64 = w_c (k-periodic)
then hw = row32*row64 (first 90 cols), prod = B * broadcast(hw), out[k] =
segment-sum of 90-wide blocks.  Raw Bass (no TileContext) with manual
semaphores: DMAs spread across engine queues, minimal fixed overhead.

Too small to shard: all 8 cores run the same replicated program (SPMD);
core 0's output is returned.
"""

from contextlib import ExitStack
from math import factorial, sqrt

import numpy as np

import concourse.bass as bass
import concourse.mybir as mybir
from concourse.bass_utils import run_bass_kernel_spmd

# ---------------------------------------------------------------------------
# Compile-time constants (Clebsch-Gordan coefficients, e3nn real basis)
# ---------------------------------------------------------------------------
INPUT_A_L = [0, 1, 2, 3, 4]
INPUT_H_L = [0, 1, 2, 3, 4]
L_OUT = 2
KO = 2 * L_OUT + 1  # 5


def _su2_cg(j1, m1, j2, m2, j3, m3):
    if m3 != m1 + m2:
        return 0.0
    f = factorial
    vmin = int(max(-j1 + j2 + m3, -j1 + m1, 0))
    vmax = int(min(j2 + j3 + m1, j3 - j1 + j2, j3 + m3))
    C = sqrt((2 * j3 + 1) * f(j3 + j1 - j2) * f(j3 - j1 + j2) * f(j1 + j2 - j3) / f(j1 + j2 + j3 + 1)
             * f(j3 + m3) * f(j3 - m3) / (f(j1 + m1) * f(j1 - m1) * f(j2 + m2) * f(j2 - m2)))
    S = 0.0
    for v in range(vmin, vmax + 1):
        S += (-1) ** (v + j2 + m2) * f(j2 + j3 + m1 - v) * f(j1 - m1 + v) / (
            f(v) * f(j3 - j1 + j2 - v) * f(j3 + m3 - v) * f(v + j1 - j2 - m3))
    return C * S


def _su2_clebsch_gordan(j1, j2, j3):
    C = np.zeros((2 * j1 + 1, 2 * j2 + 1, 2 * j3 + 1))
    for m1 in range(-j1, j1 + 1):
        for m2 in range(-j2, j2 + 1):
            m3 = m1 + m2
            if -j3 <= m3 <= j3:
                C[j1 + m1, j2 + m2, j3 + m3] = _su2_cg(j1, m1, j2, m2, j3, m3)
    return C


def _change_basis_real_to_complex(l):
    q = np.zeros((2 * l + 1, 2 * l + 1), dtype=np.complex128)
    for m in range(-l, 0):
        q[l + m, l + abs(m)] = 1.0 / sqrt(2)
        q[l + m, l - abs(m)] = -1j / sqrt(2)
    q[l, l] = 1.0
    for m in range(1, l + 1):
        q[l + m, l + abs(m)] = (-1) ** m / sqrt(2)
        q[l + m, l - abs(m)] = 1j * (-1) ** m / sqrt(2)
    return (-1j) ** l * q


def _so3_clebsch_gordan(l1, l2, l3):
    C = _su2_clebsch_gordan(l1, l2, l3).astype(np.complex128)
    Q1 = _change_basis_real_to_complex(l1)
    Q2 = _change_basis_real_to_complex(l2)
    Q3 = _change_basis_real_to_complex(l3)
    C = np.einsum('ij,kl,mn,ikn->jlm', Q1, Q2, np.conj(Q3.T), C)
    assert np.abs(C.imag).max() < 1e-10
    return C.real


def _build_combos():
    combos, cgs = [], []
    for ai, la in enumerate(INPUT_A_L):
        for hi, lh in enumerate(INPUT_H_L):
            if abs(la - lh) <= L_OUT <= la + lh:
                cg = _so3_clebsch_gordan(la, lh, L_OUT)
                if np.abs(cg).sum() > 0:
                    combos.append((ai, hi))
                    cgs.append(cg.astype(np.float32))
    return combos, cgs


VALID_COMBOS, CG_TENSORS = _build_combos()
CJ_OFF = []
NCJ = 0
for (_ai, _hi) in VALID_COMBOS:
    CJ_OFF.append(NCJ)
    NCJ += 2 * _hi + 1
NCOL = KO * NCJ  # 450


def _build_G():
    G = np.zeros((75, NCOL), dtype=np.float32)
    for c, (ai, hi) in enumerate(VALID_COMBOS):
        cg = CG_TENSORS[c]
        na, nh = 2 * ai + 1, 2 * hi + 1
        for k in range(KO):
            base = k * NCJ + CJ_OFF[c]
            G[ai * ai: ai * ai + na, base: base + nh] += cg[:, :, k]
            for j in range(nh):
                G[25 + hi * hi + j, base + j] = 1.0
                G[50 + c, base + j] = 1.0
    return G


G_CONST = _build_G()

# ---------------------------------------------------------------------------
# Bass kernel (raw, manual semaphores)
# ---------------------------------------------------------------------------
F32 = mybir.dt.float32
F16 = mybir.dt.float16
_NC = None


def build_nc():
    global _NC
    if _NC is not None:
        return _NC
    nc = bass.Bass(target_bir_lowering=False, enable_partition_id=False)

    iv = nc.dram_tensor("iv", [75, 65], F16, kind="ExternalInput")
    gm = nc.dram_tensor("gmat", [75, NCOL], F16, kind="ExternalInput")
    out_dram = nc.dram_tensor("out", [KO], F32, kind="ExternalOutput")

    with ExitStack() as ctx:
        itile = ctx.enter_context(nc.sbuf_tensor("itile", [75, 65], F16))
        ctile = ctx.enter_context(nc.sbuf_tensor("ctile", [75, NCOL], F16))
        s1 = ctx.enter_context(nc.sbuf_tensor("s1", [1, NCJ], F32))
        hw = ctx.enter_context(nc.sbuf_tensor("hw", [1, NCJ], F32))
        prod = ctx.enter_context(nc.sbuf_tensor("prod", [1, NCOL], F32))
        o5 = ctx.enter_context(nc.sbuf_tensor("o5", [1, KO], F32))
        acc = ctx.enter_context(nc.psum_tensor("acc", [65, NCOL], F32))
        s_iv = ctx.enter_context(nc.semaphore("s_iv"))
        s_g = ctx.enter_context(nc.semaphore("s_g"))
        s_mm = ctx.enter_context(nc.semaphore("s_mm"))
        s_ve = ctx.enter_context(nc.semaphore("s_ve"))
        s_out = ctx.enter_context(nc.semaphore("s_out"))
        block = ctx.enter_context(nc.Block())

        @block.sync
        def _(sync):
            sync.dma_start(itile[:], iv[:]).then_inc(s_iv, 16)
            sync.dma_start(ctile[0:25, :], gm[0:25, :]).then_inc(s_g, 16)
            sync.wait_ge(s_ve, 4)
            sync.dma_start(out_dram[:], o5[:]).then_inc(s_out, 16)
            sync.wait_ge(s_out, 16)

        @block.scalar
        def _(scalar):
            scalar.dma_start(ctile[25:50, :], gm[25:50, :]).then_inc(s_g, 16)
            scalar.dma_start(ctile[50:75, :], gm[50:75, :]).then_inc(s_g, 16)

        @block.tensor
        def _(tensor):
            tensor.wait_ge(s_iv, 16)
            tensor.wait_ge(s_g, 48)
            nc.tensor.matmul(acc[:], itile[:], ctile[:],
                             start=True, stop=True).then_inc(s_mm, 1)

        @block.vector
        def _(vector):
            nc.vector.tensor_copy(s1[:], acc[32:33, 0:NCJ]) \
                .wait_op(s_mm, 1, "sem-ge").then_inc(s_ve, 1)
            nc.vector.tensor_mul(hw[:], s1[:], acc[64:65, 0:NCJ]) \
                .wait_op(s_ve, 1, "sem-ge").then_inc(s_ve, 1)
            hw_bcast = bass.AP(hw, 0, [[NCJ, 1], [0, KO], [1, NCJ]])
            nc.vector.tensor_mul(
                prod[:].rearrange("p (a b) -> p a b", b=NCJ),
                acc[0:1, :].rearrange("p (a b) -> p a b", b=NCJ),
                hw_bcast).wait_op(s_ve, 2, "sem-ge").then_inc(s_ve, 1)
            nc.vector.reduce_sum(
                o5[:], prod[:].rearrange("p (a b) -> p a b", b=NCJ),
                axis=mybir.AxisListType.X) \
                .wait_op(s_ve, 3, "sem-ge").then_inc(s_ve, 1)

    _NC = nc
    return nc


def _pack_iv(inputs) -> np.ndarray:
    iv = np.zeros((75, 65), dtype=np.float16)
    for i, l in enumerate(INPUT_A_L):
        iv[l * l:(l + 1) * (l + 1), 0] = np.asarray(inputs[f"a{i}"], np.float16)
    for i, l in enumerate(INPUT_H_L):
        iv[25 + l * l:25 + (l + 1) * (l + 1), 32] = np.asarray(inputs[f"h{i}"], np.float16)
    iv[50:75, 64] = np.asarray(inputs["weight"], np.float16)
    return iv


def kernel(**inputs) -> np.ndarray:
    nc = build_nc()
    in_map = {"iv": _pack_iv(inputs), "gmat": G_CONST.astype(np.float16)}
    core_ids = list(range(8))
    res = run_bass_kernel_spmd(nc, [dict(in_map) for _ in core_ids], core_ids)
    return res.results[0]["out"]


# revision 11
# speedup vs baseline: 1.0027x; 1.0027x over previous
"""Trainium2 Bass kernel for nn_CGWeight: weighted Clebsch-Gordan tensor product.

out[k] = nan_to_num( sum_c w_c * sum_{i,j} CG_c[i,j,k] * A[ai_c][i] * H[hi_c][j] )

One small fp32 matmul against a precomputed [75, 450] constant G:
  lhsT [75, 65]  = host-packed block-diagonal: concat(A) rows 0-24 col 0,
                   concat(H) rows 25-49 col 32, weight rows 50-74 col 64
                   (cols 0/32/64 so result rows hit legal DVE start partitions)
  rhs  [75, 450] = G: rows 0-24 CG contraction (cols ordered (k, c, j)),
                   rows 25-49 H-scatter 0/1, rows 50-74 w-scatter 0/1
  PSUM [65, 450]: row0 = B[(k,c,j)] = sum_i CG_c[i,j,k] A_i,
                  row32 = H[hi_c][j] (k-periodic), row64 = w_c (k-periodic)
then hw = row32*row64 (first 90 cols), prod = B * broadcast(hw), out[k] =
segment-sum of 90-wide blocks.  Raw Bass (no TileContext) with manual
semaphores: DMAs spread across engine queues, minimal fixed overhead.

Too small to shard: all 8 cores run the same replicated program (SPMD);
core 0's output is returned.
"""

from contextlib import ExitStack
from math import factorial, sqrt

import numpy as np

import concourse.bass as bass
import concourse.mybir as mybir
from concourse.bass_utils import run_bass_kernel_spmd

# ---------------------------------------------------------------------------
# Compile-time constants (Clebsch-Gordan coefficients, e3nn real basis)
# ---------------------------------------------------------------------------
INPUT_A_L = [0, 1, 2, 3, 4]
INPUT_H_L = [0, 1, 2, 3, 4]
L_OUT = 2
KO = 2 * L_OUT + 1  # 5


def _su2_cg(j1, m1, j2, m2, j3, m3):
    if m3 != m1 + m2:
        return 0.0
    f = factorial
    vmin = int(max(-j1 + j2 + m3, -j1 + m1, 0))
    vmax = int(min(j2 + j3 + m1, j3 - j1 + j2, j3 + m3))
    C = sqrt((2 * j3 + 1) * f(j3 + j1 - j2) * f(j3 - j1 + j2) * f(j1 + j2 - j3) / f(j1 + j2 + j3 + 1)
             * f(j3 + m3) * f(j3 - m3) / (f(j1 + m1) * f(j1 - m1) * f(j2 + m2) * f(j2 - m2)))
    S = 0.0
    for v in range(vmin, vmax + 1):
        S += (-1) ** (v + j2 + m2) * f(j2 + j3 + m1 - v) * f(j1 - m1 + v) / (
            f(v) * f(j3 - j1 + j2 - v) * f(j3 + m3 - v) * f(v + j1 - j2 - m3))
    return C * S


def _su2_clebsch_gordan(j1, j2, j3):
    C = np.zeros((2 * j1 + 1, 2 * j2 + 1, 2 * j3 + 1))
    for m1 in range(-j1, j1 + 1):
        for m2 in range(-j2, j2 + 1):
            m3 = m1 + m2
            if -j3 <= m3 <= j3:
                C[j1 + m1, j2 + m2, j3 + m3] = _su2_cg(j1, m1, j2, m2, j3, m3)
    return C


def _change_basis_real_to_complex(l):
    q = np.zeros((2 * l + 1, 2 * l + 1), dtype=np.complex128)
    for m in range(-l, 0):
        q[l + m, l + abs(m)] = 1.0 / sqrt(2)
        q[l + m, l - abs(m)] = -1j / sqrt(2)
    q[l, l] = 1.0
    for m in range(1, l + 1):
        q[l + m, l + abs(m)] = (-1) ** m / sqrt(2)
        q[l + m, l - abs(m)] = 1j * (-1) ** m / sqrt(2)
    return (-1j) ** l * q


def _so3_clebsch_gordan(l1, l2, l3):
    C = _su2_clebsch_gordan(l1, l2, l3).astype(np.complex128)
    Q1 = _change_basis_real_to_complex(l1)
    Q2 = _change_basis_real_to_complex(l2)
    Q3 = _change_basis_real_to_complex(l3)
    C = np.einsum('ij,kl,mn,ikn->jlm', Q1, Q2, np.conj(Q3.T), C)
    assert np.abs(C.imag).max() < 1e-10
    return C.real


def _build_combos():
    combos, cgs = [], []
    for ai, la in enumerate(INPUT_A_L):
        for hi, lh in enumerate(INPUT_H_L):
            if abs(la - lh) <= L_OUT <= la + lh:
                cg = _so3_clebsch_gordan(la, lh, L_OUT)
                if np.abs(cg).sum() > 0:
                    combos.append((ai, hi))
                    cgs.append(cg.astype(np.float32))
    return combos, cgs


VALID_COMBOS, CG_TENSORS = _build_combos()
CJ_OFF = []
NCJ = 0
for (_ai, _hi) in VALID_COMBOS:
    CJ_OFF.append(NCJ)
    NCJ += 2 * _hi + 1
NCOL = KO * NCJ  # 450


def _build_G():
    G = np.zeros((75, NCOL), dtype=np.float32)
    for c, (ai, hi) in enumerate(VALID_COMBOS):
        cg = CG_TENSORS[c]
        na, nh = 2 * ai + 1, 2 * hi + 1
        for k in range(KO):
            base = k * NCJ + CJ_OFF[c]
            G[ai * ai: ai * ai + na, base: base + nh] += cg[:, :, k]
            for j in range(nh):
                G[25 + hi * hi + j, base + j] = 1.0
                G[50 + c, base + j] = 1.0
    return G


G_CONST = _build_G()

# ---------------------------------------------------------------------------
# Bass kernel (raw, manual semaphores)
# ---------------------------------------------------------------------------
F32 = mybir.dt.float32
F16 = mybir.dt.float16
_NC = None


def build_nc():
    global _NC
    if _NC is not None:
        return _NC
    nc = bass.Bass(target_bir_lowering=False, enable_partition_id=False)

    iv = nc.dram_tensor("iv", [75, 65], F16, kind="ExternalInput")
    # gmat split: rows 0-31 full width (CG rows + first 7 scatter rows,
    # zeros included); rows 32-74 only the k=0 block (cols 0-89) — the
    # rest of those rows is zeroed on-chip by two aligned memsets.
    gt = nc.dram_tensor("gtop", [32, NCOL], F16, kind="ExternalInput")
    gs = nc.dram_tensor("gsc", [43, NCJ], F16, kind="ExternalInput")
    out_dram = nc.dram_tensor("out", [KO], F32, kind="ExternalOutput")

    with ExitStack() as ctx:
        itile = ctx.enter_context(nc.sbuf_tensor("itile", [75, 65], F16))
        ctile = ctx.enter_context(nc.sbuf_tensor("ctile", [75, NCOL], F16))
        s1 = ctx.enter_context(nc.sbuf_tensor("s1", [1, NCJ], F32))
        hw = ctx.enter_context(nc.sbuf_tensor("hw", [1, NCJ], F32))
        prod = ctx.enter_context(nc.sbuf_tensor("prod", [1, NCOL], F32))
        o5 = ctx.enter_context(nc.sbuf_tensor("o5", [1, KO], F32))
        acc = ctx.enter_context(nc.psum_tensor("acc", [65, NCOL], F32))
        s_iv = ctx.enter_context(nc.semaphore("s_iv"))
        s_g = ctx.enter_context(nc.semaphore("s_g"))
        s_g2 = ctx.enter_context(nc.semaphore("s_g2"))
        s_ms = ctx.enter_context(nc.semaphore("s_ms"))
        s_mm = ctx.enter_context(nc.semaphore("s_mm"))
        s_ve = ctx.enter_context(nc.semaphore("s_ve"))
        s_out = ctx.enter_context(nc.semaphore("s_out"))
        block = ctx.enter_context(nc.Block())

        @block.sync
        def _(sync):
            sync.dma_start(ctile[0:16, :], gt[0:16, :]).then_inc(s_g, 16)
            sync.dma_start(itile[:], iv[:]).then_inc(s_iv, 16)
            sync.wait_ge(s_ve, 4)
            sync.dma_start(out_dram[:], o5[:]).then_inc(s_out, 16)
            sync.wait_ge(s_out, 16)

        @block.scalar
        def _(scalar):
            scalar.dma_start(ctile[32:75, 0:NCJ], gs[:]).then_inc(s_g, 16)

        @block.gpsimd
        def _(gpsimd):
            gpsimd.dma_start(ctile[16:32, :], gt[16:32, :]).then_inc(s_g2, 16)
            gpsimd.memset(ctile[64:75, NCJ:NCOL], 0.0).then_inc(s_ms, 1)

        @block.tensor
        def _(tensor):
            tensor.wait_ge(s_iv, 16)
            tensor.wait_ge(s_g, 32)
            tensor.wait_ge(s_g2, 16)
            tensor.wait_ge(s_ms, 2)
            nc.tensor.matmul(acc[:], itile[:], ctile[:],
                             start=True, stop=True).then_inc(s_mm, 1)

        @block.vector
        def _(vector):
            nc.vector.memset(ctile[32:64, NCJ:NCOL], 0.0).then_inc(s_ms, 1)
            nc.vector.tensor_copy(s1[:], acc[32:33, 0:NCJ]) \
                .wait_op(s_mm, 1, "sem-ge").then_inc(s_ve, 1)
            nc.vector.tensor_mul(hw[:], s1[:], acc[64:65, 0:NCJ]) \
                .wait_op(s_ve, 1, "sem-ge").then_inc(s_ve, 1)
            hw_bcast = bass.AP(hw, 0, [[NCJ, 1], [0, KO], [1, NCJ]])
            nc.vector.tensor_mul(
                prod[:].rearrange("p (a b) -> p a b", b=NCJ),
                acc[0:1, :].rearrange("p (a b) -> p a b", b=NCJ),
                hw_bcast).wait_op(s_ve, 2, "sem-ge").then_inc(s_ve, 1)
            nc.vector.reduce_sum(
                o5[:], prod[:].rearrange("p (a b) -> p a b", b=NCJ),
                axis=mybir.AxisListType.X) \
                .wait_op(s_ve, 3, "sem-ge").then_inc(s_ve, 1)

    _NC = nc
    return nc


def _pack_iv(inputs) -> np.ndarray:
    iv = np.zeros((75, 65), dtype=np.float16)
    for i, l in enumerate(INPUT_A_L):
        iv[l * l:(l + 1) * (l + 1), 0] = np.asarray(inputs[f"a{i}"], np.float16)
    for i, l in enumerate(INPUT_H_L):
        iv[25 + l * l:25 + (l + 1) * (l + 1), 32] = np.asarray(inputs[f"h{i}"], np.float16)
    iv[50:75, 64] = np.asarray(inputs["weight"], np.float16)
    return iv


def kernel(**inputs) -> np.ndarray:
    nc = build_nc()
    g16 = G_CONST.astype(np.float16)
    in_map = {"iv": _pack_iv(inputs),
              "gtop": np.ascontiguousarray(g16[0:32, :]),
              "gsc": np.ascontiguousarray(g16[32:75, 0:NCJ])}
    core_ids = list(range(8))
    res = run_bass_kernel_spmd(nc, [dict(in_map) for _ in core_ids], core_ids)
    return res.results[0]["out"]


# revision 13
# speedup vs baseline: 1.0196x; 1.0169x over previous
"""Trainium2 Bass kernel for nn_CGWeight: weighted Clebsch-Gordan tensor product.

out[k] = nan_to_num( sum_c w_c * sum_{i,j} CG_c[i,j,k] * A[ai_c][i] * H[hi_c][j] )

One small fp32 matmul against a precomputed [75, 450] constant G:
  lhsT [75, 65]  = host-packed block-diagonal: concat(A) rows 0-24 col 0,
                   concat(H) rows 25-49 col 32, weight rows 50-74 col 64
                   (cols 0/32/64 so result rows hit legal DVE start partitions)
  rhs  [75, 450] = G: rows 0-24 CG contraction (cols ordered (k, c, j)),
                   rows 25-49 H-scatter 0/1, rows 50-74 w-scatter 0/1
  PSUM [65, 450]: row0 = B[(k,c,j)] = sum_i CG_c[i,j,k] A_i,
                  row32 = H[hi_c][j] (k-periodic), row64 = w_c (k-periodic)
then hw = row32*row64 (first 90 cols), prod = B * broadcast(hw), out[k] =
segment-sum of 90-wide blocks.  Raw Bass (no TileContext) with manual
semaphores: DMAs spread across engine queues, minimal fixed overhead.

Too small to shard: all 8 cores run the same replicated program (SPMD);
core 0's output is returned.
"""

from contextlib import ExitStack
from math import factorial, sqrt

import numpy as np

import concourse.bass as bass
import concourse.mybir as mybir
from concourse.bass_utils import run_bass_kernel_spmd

# ---------------------------------------------------------------------------
# Compile-time constants (Clebsch-Gordan coefficients, e3nn real basis)
# ---------------------------------------------------------------------------
INPUT_A_L = [0, 1, 2, 3, 4]
INPUT_H_L = [0, 1, 2, 3, 4]
L_OUT = 2
KO = 2 * L_OUT + 1  # 5


def _su2_cg(j1, m1, j2, m2, j3, m3):
    if m3 != m1 + m2:
        return 0.0
    f = factorial
    vmin = int(max(-j1 + j2 + m3, -j1 + m1, 0))
    vmax = int(min(j2 + j3 + m1, j3 - j1 + j2, j3 + m3))
    C = sqrt((2 * j3 + 1) * f(j3 + j1 - j2) * f(j3 - j1 + j2) * f(j1 + j2 - j3) / f(j1 + j2 + j3 + 1)
             * f(j3 + m3) * f(j3 - m3) / (f(j1 + m1) * f(j1 - m1) * f(j2 + m2) * f(j2 - m2)))
    S = 0.0
    for v in range(vmin, vmax + 1):
        S += (-1) ** (v + j2 + m2) * f(j2 + j3 + m1 - v) * f(j1 - m1 + v) / (
            f(v) * f(j3 - j1 + j2 - v) * f(j3 + m3 - v) * f(v + j1 - j2 - m3))
    return C * S


def _su2_clebsch_gordan(j1, j2, j3):
    C = np.zeros((2 * j1 + 1, 2 * j2 + 1, 2 * j3 + 1))
    for m1 in range(-j1, j1 + 1):
        for m2 in range(-j2, j2 + 1):
            m3 = m1 + m2
            if -j3 <= m3 <= j3:
                C[j1 + m1, j2 + m2, j3 + m3] = _su2_cg(j1, m1, j2, m2, j3, m3)
    return C


def _change_basis_real_to_complex(l):
    q = np.zeros((2 * l + 1, 2 * l + 1), dtype=np.complex128)
    for m in range(-l, 0):
        q[l + m, l + abs(m)] = 1.0 / sqrt(2)
        q[l + m, l - abs(m)] = -1j / sqrt(2)
    q[l, l] = 1.0
    for m in range(1, l + 1):
        q[l + m, l + abs(m)] = (-1) ** m / sqrt(2)
        q[l + m, l - abs(m)] = 1j * (-1) ** m / sqrt(2)
    return (-1j) ** l * q


def _so3_clebsch_gordan(l1, l2, l3):
    C = _su2_clebsch_gordan(l1, l2, l3).astype(np.complex128)
    Q1 = _change_basis_real_to_complex(l1)
    Q2 = _change_basis_real_to_complex(l2)
    Q3 = _change_basis_real_to_complex(l3)
    C = np.einsum('ij,kl,mn,ikn->jlm', Q1, Q2, np.conj(Q3.T), C)
    assert np.abs(C.imag).max() < 1e-10
    return C.real


def _build_combos():
    combos, cgs = [], []
    for ai, la in enumerate(INPUT_A_L):
        for hi, lh in enumerate(INPUT_H_L):
            if abs(la - lh) <= L_OUT <= la + lh:
                cg = _so3_clebsch_gordan(la, lh, L_OUT)
                if np.abs(cg).sum() > 0:
                    combos.append((ai, hi))
                    cgs.append(cg.astype(np.float32))
    return combos, cgs


VALID_COMBOS, CG_TENSORS = _build_combos()
CJ_OFF = []
NCJ = 0
for (_ai, _hi) in VALID_COMBOS:
    CJ_OFF.append(NCJ)
    NCJ += 2 * _hi + 1
NCOL = KO * NCJ  # 450


def _build_G():
    G = np.zeros((75, NCOL), dtype=np.float32)
    for c, (ai, hi) in enumerate(VALID_COMBOS):
        cg = CG_TENSORS[c]
        na, nh = 2 * ai + 1, 2 * hi + 1
        for k in range(KO):
            base = k * NCJ + CJ_OFF[c]
            G[ai * ai: ai * ai + na, base: base + nh] += cg[:, :, k]
            for j in range(nh):
                G[25 + hi * hi + j, base + j] = 1.0
                G[50 + c, base + j] = 1.0
    return G


G_CONST = _build_G()

# ---------------------------------------------------------------------------
# Bass kernel (raw, manual semaphores)
# ---------------------------------------------------------------------------
F32 = mybir.dt.float32
F16 = mybir.dt.float16
_NC = None


def build_nc():
    global _NC
    if _NC is not None:
        return _NC
    nc = bass.Bass(target_bir_lowering=False, enable_partition_id=False)

    iv = nc.dram_tensor("iv", [75, 65], F16, kind="ExternalInput")
    # gmat split: rows 0-31 full width (CG rows + first 7 scatter rows,
    # zeros included); rows 32-74 only the k=0 block (cols 0-89) — the
    # rest of those rows is zeroed on-chip by two aligned memsets.
    gt = nc.dram_tensor("gtop", [32, NCOL], F16, kind="ExternalInput")
    gs = nc.dram_tensor("gsc", [43, NCJ], F16, kind="ExternalInput")
    out_dram = nc.dram_tensor("out", [KO], F32, kind="ExternalOutput")

    with ExitStack() as ctx:
        # one SBUF tensor: cols 0-64 = lhsT (block-diag inputs), 65.. = rhs G
        big = ctx.enter_context(nc.sbuf_tensor("big", [75, 65 + NCOL], F16))
        s1 = ctx.enter_context(nc.sbuf_tensor("s1", [1, NCJ], F32))
        hw = ctx.enter_context(nc.sbuf_tensor("hw", [1, NCJ], F32))
        prod = ctx.enter_context(nc.sbuf_tensor("prod", [1, NCOL], F32))
        o5 = ctx.enter_context(nc.sbuf_tensor("o5", [1, KO], F32))
        acc = ctx.enter_context(nc.psum_tensor("acc", [65, NCOL], F32))
        s_g = ctx.enter_context(nc.semaphore("s_g"))
        s_g2 = ctx.enter_context(nc.semaphore("s_g2"))
        s_ms = ctx.enter_context(nc.semaphore("s_ms"))
        s_mm = ctx.enter_context(nc.semaphore("s_mm"))
        s_ve = ctx.enter_context(nc.semaphore("s_ve"))
        s_out = ctx.enter_context(nc.semaphore("s_out"))
        block = ctx.enter_context(nc.Block())

        @block.sync
        def _(sync):
            sync.dma_start(big[:, 0:65], iv[:]).then_inc(s_g, 16)
            sync.wait_ge(s_ve, 4)
            sync.dma_start(out_dram[:], o5[:]).then_inc(s_out, 16)
            sync.wait_ge(s_out, 16)

        @block.scalar
        def _(scalar):
            scalar.dma_start(big[0:32, 65:65 + NCOL], gt[:]).then_inc(s_g, 16)

        @block.gpsimd
        def _(gpsimd):
            gpsimd.dma_start(big[32:75, 65:65 + NCJ], gs[:]).then_inc(s_g2, 16)
            gpsimd.memset(big[64:75, 65 + NCJ:65 + NCOL], 0.0).then_inc(s_ms, 1)

        @block.tensor
        def _(tensor):
            tensor.wait_ge(s_g, 32).wait_op(s_ms, 2, "sem-ge", check=False)
            nc.tensor.matmul(acc[:], big[:, 0:65], big[:, 65:65 + NCOL],
                             start=True, stop=True) \
                .wait_op(s_g2, 16, "sem-ge").then_inc(s_mm, 1)

        @block.vector
        def _(vector):
            nc.vector.memset(big[32:64, 65 + NCJ:65 + NCOL], 0.0).then_inc(s_ms, 1)
            nc.vector.tensor_copy(s1[:], acc[32:33, 0:NCJ]) \
                .wait_op(s_mm, 1, "sem-ge").then_inc(s_ve, 1)
            nc.vector.tensor_mul(hw[:], s1[:], acc[64:65, 0:NCJ]) \
                .wait_op(s_ve, 1, "sem-ge").then_inc(s_ve, 1)
            hw_bcast = bass.AP(hw, 0, [[NCJ, 1], [0, KO], [1, NCJ]])
            nc.vector.tensor_mul(
                prod[:].rearrange("p (a b) -> p a b", b=NCJ),
                acc[0:1, :].rearrange("p (a b) -> p a b", b=NCJ),
                hw_bcast).wait_op(s_ve, 2, "sem-ge").then_inc(s_ve, 1)
            nc.vector.reduce_sum(
                o5[:], prod[:].rearrange("p (a b) -> p a b", b=NCJ),
                axis=mybir.AxisListType.X) \
                .wait_op(s_ve, 3, "sem-ge").then_inc(s_ve, 1)

    _NC = nc
    return nc


def _pack_iv(inputs) -> np.ndarray:
    iv = np.zeros((75, 65), dtype=np.float16)
    for i, l in enumerate(INPUT_A_L):
        iv[l * l:(l + 1) * (l + 1), 0] = np.asarray(inputs[f"a{i}"], np.float16)
    for i, l in enumerate(INPUT_H_L):
        iv[25 + l * l:25 + (l + 1) * (l + 1), 32] = np.asarray(inputs[f"h{i}"], np.float16)
    iv[50:75, 64] = np.asarray(inputs["weight"], np.float16)
    return iv


def kernel(**inputs) -> np.ndarray:
    nc = build_nc()
    g16 = G_CONST.astype(np.float16)
    in_map = {"iv": _pack_iv(inputs),
              "gtop": np.ascontiguousarray(g16[0:32, :]),
              "gsc": np.ascontiguousarray(g16[32:75, 0:NCJ])}
    core_ids = list(range(8))
    res = run_bass_kernel_spmd(nc, [dict(in_map) for _ in core_ids], core_ids)
    return res.results[0]["out"]


# revision 14
# speedup vs baseline: 1.0283x; 1.0086x over previous
"""Trainium2 Bass kernel for nn_CGWeight: weighted Clebsch-Gordan tensor product.

out[k] = nan_to_num( sum_c w_c * sum_{i,j} CG_c[i,j,k] * A[ai_c][i] * H[hi_c][j] )

One small fp32 matmul against a precomputed [75, 450] constant G:
  lhsT [75, 65]  = host-packed block-diagonal: concat(A) rows 0-24 col 0,
                   concat(H) rows 25-49 col 32, weight rows 50-74 col 64
                   (cols 0/32/64 so result rows hit legal DVE start partitions)
  rhs  [75, 450] = G: rows 0-24 CG contraction (cols ordered (k, c, j)),
                   rows 25-49 H-scatter 0/1, rows 50-74 w-scatter 0/1
  PSUM [65, 450]: row0 = B[(k,c,j)] = sum_i CG_c[i,j,k] A_i,
                  row32 = H[hi_c][j] (k-periodic), row64 = w_c (k-periodic)
then hw = row32*row64 (first 90 cols), prod = B * broadcast(hw), out[k] =
segment-sum of 90-wide blocks.  Raw Bass (no TileContext) with manual
semaphores: DMAs spread across engine queues, minimal fixed overhead.

Too small to shard: all 8 cores run the same replicated program (SPMD);
core 0's output is returned.
"""

from contextlib import ExitStack
from math import factorial, sqrt

import numpy as np

import concourse.bass as bass
import concourse.mybir as mybir
from concourse.bass_utils import run_bass_kernel_spmd

# ---------------------------------------------------------------------------
# Compile-time constants (Clebsch-Gordan coefficients, e3nn real basis)
# ---------------------------------------------------------------------------
INPUT_A_L = [0, 1, 2, 3, 4]
INPUT_H_L = [0, 1, 2, 3, 4]
L_OUT = 2
KO = 2 * L_OUT + 1  # 5


def _su2_cg(j1, m1, j2, m2, j3, m3):
    if m3 != m1 + m2:
        return 0.0
    f = factorial
    vmin = int(max(-j1 + j2 + m3, -j1 + m1, 0))
    vmax = int(min(j2 + j3 + m1, j3 - j1 + j2, j3 + m3))
    C = sqrt((2 * j3 + 1) * f(j3 + j1 - j2) * f(j3 - j1 + j2) * f(j1 + j2 - j3) / f(j1 + j2 + j3 + 1)
             * f(j3 + m3) * f(j3 - m3) / (f(j1 + m1) * f(j1 - m1) * f(j2 + m2) * f(j2 - m2)))
    S = 0.0
    for v in range(vmin, vmax + 1):
        S += (-1) ** (v + j2 + m2) * f(j2 + j3 + m1 - v) * f(j1 - m1 + v) / (
            f(v) * f(j3 - j1 + j2 - v) * f(j3 + m3 - v) * f(v + j1 - j2 - m3))
    return C * S


def _su2_clebsch_gordan(j1, j2, j3):
    C = np.zeros((2 * j1 + 1, 2 * j2 + 1, 2 * j3 + 1))
    for m1 in range(-j1, j1 + 1):
        for m2 in range(-j2, j2 + 1):
            m3 = m1 + m2
            if -j3 <= m3 <= j3:
                C[j1 + m1, j2 + m2, j3 + m3] = _su2_cg(j1, m1, j2, m2, j3, m3)
    return C


def _change_basis_real_to_complex(l):
    q = np.zeros((2 * l + 1, 2 * l + 1), dtype=np.complex128)
    for m in range(-l, 0):
        q[l + m, l + abs(m)] = 1.0 / sqrt(2)
        q[l + m, l - abs(m)] = -1j / sqrt(2)
    q[l, l] = 1.0
    for m in range(1, l + 1):
        q[l + m, l + abs(m)] = (-1) ** m / sqrt(2)
        q[l + m, l - abs(m)] = 1j * (-1) ** m / sqrt(2)
    return (-1j) ** l * q


def _so3_clebsch_gordan(l1, l2, l3):
    C = _su2_clebsch_gordan(l1, l2, l3).astype(np.complex128)
    Q1 = _change_basis_real_to_complex(l1)
    Q2 = _change_basis_real_to_complex(l2)
    Q3 = _change_basis_real_to_complex(l3)
    C = np.einsum('ij,kl,mn,ikn->jlm', Q1, Q2, np.conj(Q3.T), C)
    assert np.abs(C.imag).max() < 1e-10
    return C.real


def _build_combos():
    combos, cgs = [], []
    for ai, la in enumerate(INPUT_A_L):
        for hi, lh in enumerate(INPUT_H_L):
            if abs(la - lh) <= L_OUT <= la + lh:
                cg = _so3_clebsch_gordan(la, lh, L_OUT)
                if np.abs(cg).sum() > 0:
                    combos.append((ai, hi))
                    cgs.append(cg.astype(np.float32))
    return combos, cgs


VALID_COMBOS, CG_TENSORS = _build_combos()
CJ_OFF = []
NCJ = 0
for (_ai, _hi) in VALID_COMBOS:
    CJ_OFF.append(NCJ)
    NCJ += 2 * _hi + 1
NCOL = KO * NCJ  # 450


def _build_G():
    G = np.zeros((75, NCOL), dtype=np.float32)
    for c, (ai, hi) in enumerate(VALID_COMBOS):
        cg = CG_TENSORS[c]
        na, nh = 2 * ai + 1, 2 * hi + 1
        for k in range(KO):
            base = k * NCJ + CJ_OFF[c]
            G[ai * ai: ai * ai + na, base: base + nh] += cg[:, :, k]
            for j in range(nh):
                G[25 + hi * hi + j, base + j] = 1.0
                G[50 + c, base + j] = 1.0
    return G


G_CONST = _build_G()

# ---------------------------------------------------------------------------
# Bass kernel (raw, manual semaphores)
# ---------------------------------------------------------------------------
F32 = mybir.dt.float32
F16 = mybir.dt.float16
_NC = None


def build_nc():
    global _NC
    if _NC is not None:
        return _NC
    nc = bass.Bass(target_bir_lowering=False, enable_partition_id=False)

    # one packed input: cols 0-64 = block-diag A/H/w, cols 65.. = G (fp16)
    iv = nc.dram_tensor("iv", [75, 65 + NCOL], F16, kind="ExternalInput")
    out_dram = nc.dram_tensor("out", [KO], F32, kind="ExternalOutput")

    with ExitStack() as ctx:
        # one SBUF tensor: cols 0-64 = lhsT (block-diag inputs), 65.. = rhs G
        big = ctx.enter_context(nc.sbuf_tensor("big", [75, 65 + NCOL], F16))
        s1 = ctx.enter_context(nc.sbuf_tensor("s1", [1, NCJ], F32))
        hw = ctx.enter_context(nc.sbuf_tensor("hw", [1, NCJ], F32))
        prod = ctx.enter_context(nc.sbuf_tensor("prod", [1, NCOL], F32))
        o5 = ctx.enter_context(nc.sbuf_tensor("o5", [1, KO], F32))
        acc = ctx.enter_context(nc.psum_tensor("acc", [65, NCOL], F32))
        s_g = ctx.enter_context(nc.semaphore("s_g"))
        s_ve = ctx.enter_context(nc.semaphore("s_ve"))
        block = ctx.enter_context(nc.Block())

        @block.sync
        def _(sync):
            sync.dma_start(big[0:25, :], iv[0:25, :]).then_inc(s_g, 16)
            sync.dma_start(big[25:50, :], iv[25:50, :]).then_inc(s_g, 16)
            sync.wait_ge(s_ve, 5)
            sync.dma_start(out_dram[:], o5[:]).then_inc(s_g, 16)
            sync.wait_ge(s_g, 64)

        @block.scalar
        def _(scalar):
            scalar.dma_start(big[50:75, :], iv[50:75, :]).then_inc(s_g, 16)

        @block.tensor
        def _(tensor):
            nc.tensor.matmul(acc[:], big[:, 0:65], big[:, 65:65 + NCOL],
                             start=True, stop=True) \
                .wait_op(s_g, 48, "sem-ge").then_inc(s_ve, 1)

        @block.vector
        def _(vector):
            nc.vector.tensor_copy(s1[:], acc[32:33, 0:NCJ]) \
                .wait_op(s_ve, 1, "sem-ge").then_inc(s_ve, 1)
            nc.vector.tensor_mul(hw[:], s1[:], acc[64:65, 0:NCJ]) \
                .wait_op(s_ve, 2, "sem-ge").then_inc(s_ve, 1)
            hw_bcast = bass.AP(hw, 0, [[NCJ, 1], [0, KO], [1, NCJ]])
            nc.vector.tensor_mul(
                prod[:].rearrange("p (a b) -> p a b", b=NCJ),
                acc[0:1, :].rearrange("p (a b) -> p a b", b=NCJ),
                hw_bcast).wait_op(s_ve, 3, "sem-ge").then_inc(s_ve, 1)
            nc.vector.reduce_sum(
                o5[:], prod[:].rearrange("p (a b) -> p a b", b=NCJ),
                axis=mybir.AxisListType.X) \
                .wait_op(s_ve, 4, "sem-ge").then_inc(s_ve, 1)

    _NC = nc
    return nc


_G16PAD = np.zeros((75, NCOL), dtype=np.float16)
_G16PAD[:] = G_CONST.astype(np.float16)


def _pack_iv(inputs) -> np.ndarray:
    iv = np.zeros((75, 65 + NCOL), dtype=np.float16)
    iv[:, 65:] = _G16PAD
    for i, l in enumerate(INPUT_A_L):
        iv[l * l:(l + 1) * (l + 1), 0] = np.asarray(inputs[f"a{i}"], np.float16)
    for i, l in enumerate(INPUT_H_L):
        iv[25 + l * l:25 + (l + 1) * (l + 1), 32] = np.asarray(inputs[f"h{i}"], np.float16)
    iv[50:75, 64] = np.asarray(inputs["weight"], np.float16)
    return iv


def kernel(**inputs) -> np.ndarray:
    nc = build_nc()
    in_map = {"iv": _pack_iv(inputs)}
    core_ids = list(range(8))
    res = run_bass_kernel_spmd(nc, [dict(in_map) for _ in core_ids], core_ids)
    return res.results[0]["out"]


# revision 15
# speedup vs baseline: 1.0789x; 1.0492x over previous
"""Trainium2 Bass kernel for nn_CGWeight: weighted Clebsch-Gordan tensor product.

out[k] = nan_to_num( sum_c w_c * sum_{i,j} CG_c[i,j,k] * A[ai_c][i] * H[hi_c][j] )

One small fp32 matmul against a precomputed [75, 450] constant G:
  lhsT [75, 65]  = host-packed block-diagonal: concat(A) rows 0-24 col 0,
                   concat(H) rows 25-49 col 32, weight rows 50-74 col 64
                   (cols 0/32/64 so result rows hit legal DVE start partitions)
  rhs  [75, 450] = G: rows 0-24 CG contraction (cols ordered (k, c, j)),
                   rows 25-49 H-scatter 0/1, rows 50-74 w-scatter 0/1
  PSUM [65, 450]: row0 = B[(k,c,j)] = sum_i CG_c[i,j,k] A_i,
                  row32 = H[hi_c][j] (k-periodic), row64 = w_c (k-periodic)
then hw = row32*row64 (first 90 cols), prod = B * broadcast(hw), out[k] =
segment-sum of 90-wide blocks.  Raw Bass (no TileContext) with manual
semaphores: DMAs spread across engine queues, minimal fixed overhead.

Too small to shard: all 8 cores run the same replicated program (SPMD);
core 0's output is returned.
"""

from contextlib import ExitStack
from math import factorial, sqrt

import numpy as np

import concourse.bass as bass
import concourse.mybir as mybir
from concourse.bass_utils import run_bass_kernel_spmd

# ---------------------------------------------------------------------------
# Compile-time constants (Clebsch-Gordan coefficients, e3nn real basis)
# ---------------------------------------------------------------------------
INPUT_A_L = [0, 1, 2, 3, 4]
INPUT_H_L = [0, 1, 2, 3, 4]
L_OUT = 2
KO = 2 * L_OUT + 1  # 5


def _su2_cg(j1, m1, j2, m2, j3, m3):
    if m3 != m1 + m2:
        return 0.0
    f = factorial
    vmin = int(max(-j1 + j2 + m3, -j1 + m1, 0))
    vmax = int(min(j2 + j3 + m1, j3 - j1 + j2, j3 + m3))
    C = sqrt((2 * j3 + 1) * f(j3 + j1 - j2) * f(j3 - j1 + j2) * f(j1 + j2 - j3) / f(j1 + j2 + j3 + 1)
             * f(j3 + m3) * f(j3 - m3) / (f(j1 + m1) * f(j1 - m1) * f(j2 + m2) * f(j2 - m2)))
    S = 0.0
    for v in range(vmin, vmax + 1):
        S += (-1) ** (v + j2 + m2) * f(j2 + j3 + m1 - v) * f(j1 - m1 + v) / (
            f(v) * f(j3 - j1 + j2 - v) * f(j3 + m3 - v) * f(v + j1 - j2 - m3))
    return C * S


def _su2_clebsch_gordan(j1, j2, j3):
    C = np.zeros((2 * j1 + 1, 2 * j2 + 1, 2 * j3 + 1))
    for m1 in range(-j1, j1 + 1):
        for m2 in range(-j2, j2 + 1):
            m3 = m1 + m2
            if -j3 <= m3 <= j3:
                C[j1 + m1, j2 + m2, j3 + m3] = _su2_cg(j1, m1, j2, m2, j3, m3)
    return C


def _change_basis_real_to_complex(l):
    q = np.zeros((2 * l + 1, 2 * l + 1), dtype=np.complex128)
    for m in range(-l, 0):
        q[l + m, l + abs(m)] = 1.0 / sqrt(2)
        q[l + m, l - abs(m)] = -1j / sqrt(2)
    q[l, l] = 1.0
    for m in range(1, l + 1):
        q[l + m, l + abs(m)] = (-1) ** m / sqrt(2)
        q[l + m, l - abs(m)] = 1j * (-1) ** m / sqrt(2)
    return (-1j) ** l * q


def _so3_clebsch_gordan(l1, l2, l3):
    C = _su2_clebsch_gordan(l1, l2, l3).astype(np.complex128)
    Q1 = _change_basis_real_to_complex(l1)
    Q2 = _change_basis_real_to_complex(l2)
    Q3 = _change_basis_real_to_complex(l3)
    C = np.einsum('ij,kl,mn,ikn->jlm', Q1, Q2, np.conj(Q3.T), C)
    assert np.abs(C.imag).max() < 1e-10
    return C.real


def _build_combos():
    combos, cgs = [], []
    for ai, la in enumerate(INPUT_A_L):
        for hi, lh in enumerate(INPUT_H_L):
            if abs(la - lh) <= L_OUT <= la + lh:
                cg = _so3_clebsch_gordan(la, lh, L_OUT)
                if np.abs(cg).sum() > 0:
                    combos.append((ai, hi))
                    cgs.append(cg.astype(np.float32))
    return combos, cgs


VALID_COMBOS, CG_TENSORS = _build_combos()
CJ_OFF = []
NCJ = 0
for (_ai, _hi) in VALID_COMBOS:
    CJ_OFF.append(NCJ)
    NCJ += 2 * _hi + 1
NCOL = KO * NCJ  # 450


def _build_G():
    G = np.zeros((75, NCOL), dtype=np.float32)
    for c, (ai, hi) in enumerate(VALID_COMBOS):
        cg = CG_TENSORS[c]
        na, nh = 2 * ai + 1, 2 * hi + 1
        for k in range(KO):
            base = k * NCJ + CJ_OFF[c]
            G[ai * ai: ai * ai + na, base: base + nh] += cg[:, :, k]
            for j in range(nh):
                G[25 + hi * hi + j, base + j] = 1.0
                G[50 + c, base + j] = 1.0
    return G


G_CONST = _build_G()

# ---------------------------------------------------------------------------
# Bass kernel (raw, manual semaphores)
# ---------------------------------------------------------------------------
F32 = mybir.dt.float32
F16 = mybir.dt.float16
_NC = None


def build_nc():
    global _NC
    if _NC is not None:
        return _NC
    nc = bass.Bass(target_bir_lowering=False, enable_partition_id=False)

    # one packed input: cols 0-64 = block-diag A/H/w, cols 65.. = G (fp16)
    iv = nc.dram_tensor("iv", [75, 65 + NCOL], F16, kind="ExternalInput")
    out_dram = nc.dram_tensor("out", [KO], F32, kind="ExternalOutput")

    with ExitStack() as ctx:
        # one SBUF tensor: cols 0-64 = lhsT (block-diag inputs), 65.. = rhs G
        big = ctx.enter_context(nc.sbuf_tensor("big", [75, 65 + NCOL], F16))
        s1 = ctx.enter_context(nc.sbuf_tensor("s1", [1, NCJ], F32))
        hw = ctx.enter_context(nc.sbuf_tensor("hw", [1, NCJ], F32))
        prod = ctx.enter_context(nc.sbuf_tensor("prod", [1, NCOL], F32))
        o5 = ctx.enter_context(nc.sbuf_tensor("o5", [1, KO], F32))
        acc = ctx.enter_context(nc.psum_tensor("acc", [65, NCOL], F32))
        s_g = ctx.enter_context(nc.semaphore("s_g"))
        s_ve = ctx.enter_context(nc.semaphore("s_ve"))
        block = ctx.enter_context(nc.Block())

        @block.sync
        def _(sync):
            sync.dma_start(big[0:25, :], iv[0:25, :]).then_inc(s_g, 16)
            sync.dma_start(big[25:50, :], iv[25:50, :]).then_inc(s_g, 16)
            sync.wait_ge(s_ve, 5)
            sync.dma_start(out_dram[:], o5[:]).then_inc(s_g, 16)

        @block.scalar
        def _(scalar):
            scalar.dma_start(big[50:75, :], iv[50:75, :]).then_inc(s_g, 16)

        @block.tensor
        def _(tensor):
            nc.tensor.matmul(acc[:], big[:, 0:65], big[:, 65:65 + NCOL],
                             start=True, stop=True) \
                .wait_op(s_g, 48, "sem-ge").then_inc(s_ve, 1)

        @block.vector
        def _(vector):
            nc.vector.tensor_copy(s1[:], acc[32:33, 0:NCJ]) \
                .wait_op(s_ve, 1, "sem-ge").then_inc(s_ve, 1)
            nc.vector.tensor_mul(hw[:], s1[:], acc[64:65, 0:NCJ]) \
                .wait_op(s_ve, 2, "sem-ge").then_inc(s_ve, 1)
            hw_bcast = bass.AP(hw, 0, [[NCJ, 1], [0, KO], [1, NCJ]])
            nc.vector.tensor_mul(
                prod[:].rearrange("p (a b) -> p a b", b=NCJ),
                acc[0:1, :].rearrange("p (a b) -> p a b", b=NCJ),
                hw_bcast).wait_op(s_ve, 3, "sem-ge").then_inc(s_ve, 1)
            nc.vector.reduce_sum(
                o5[:], prod[:].rearrange("p (a b) -> p a b", b=NCJ),
                axis=mybir.AxisListType.X) \
                .wait_op(s_ve, 4, "sem-ge").then_inc(s_ve, 1)

    _NC = nc
    return nc


_G16PAD = np.zeros((75, NCOL), dtype=np.float16)
_G16PAD[:] = G_CONST.astype(np.float16)


def _pack_iv(inputs) -> np.ndarray:
    iv = np.zeros((75, 65 + NCOL), dtype=np.float16)
    iv[:, 65:] = _G16PAD
    for i, l in enumerate(INPUT_A_L):
        iv[l * l:(l + 1) * (l + 1), 0] = np.asarray(inputs[f"a{i}"], np.float16)
    for i, l in enumerate(INPUT_H_L):
        iv[25 + l * l:25 + (l + 1) * (l + 1), 32] = np.asarray(inputs[f"h{i}"], np.float16)
    iv[50:75, 64] = np.asarray(inputs["weight"], np.float16)
    return iv


def kernel(**inputs) -> np.ndarray:
    nc = build_nc()
    in_map = {"iv": _pack_iv(inputs)}
    core_ids = list(range(8))
    res = run_bass_kernel_spmd(nc, [dict(in_map) for _ in core_ids], core_ids)
    return res.results[0]["out"]


# revision 16
# speedup vs baseline: 1.0821x; 1.0030x over previous
"""Trainium2 Bass kernel for nn_CGWeight: weighted Clebsch-Gordan tensor product.

out[k] = nan_to_num( sum_c w_c * sum_{i,j} CG_c[i,j,k] * A[ai_c][i] * H[hi_c][j] )

One small fp32 matmul against a precomputed [75, 450] constant G:
  lhsT [75, 65]  = host-packed block-diagonal: concat(A) rows 0-24 col 0,
                   concat(H) rows 25-49 col 32, weight rows 50-74 col 64
                   (cols 0/32/64 so result rows hit legal DVE start partitions)
  rhs  [75, 450] = G: rows 0-24 CG contraction (cols ordered (k, c, j)),
                   rows 25-49 H-scatter 0/1, rows 50-74 w-scatter 0/1
  PSUM [65, 450]: row0 = B[(k,c,j)] = sum_i CG_c[i,j,k] A_i,
                  row32 = H[hi_c][j] (k-periodic), row64 = w_c (k-periodic)
then hw = row32*row64 (first 90 cols), prod = B * broadcast(hw), out[k] =
segment-sum of 90-wide blocks.  Raw Bass (no TileContext) with manual
semaphores: DMAs spread across engine queues, minimal fixed overhead.

Too small to shard: all 8 cores run the same replicated program (SPMD);
core 0's output is returned.
"""

from contextlib import ExitStack
from math import factorial, sqrt

import numpy as np

import concourse.bass as bass
import concourse.mybir as mybir
from concourse.bass_utils import run_bass_kernel_spmd

# ---------------------------------------------------------------------------
# Compile-time constants (Clebsch-Gordan coefficients, e3nn real basis)
# ---------------------------------------------------------------------------
INPUT_A_L = [0, 1, 2, 3, 4]
INPUT_H_L = [0, 1, 2, 3, 4]
L_OUT = 2
KO = 2 * L_OUT + 1  # 5


def _su2_cg(j1, m1, j2, m2, j3, m3):
    if m3 != m1 + m2:
        return 0.0
    f = factorial
    vmin = int(max(-j1 + j2 + m3, -j1 + m1, 0))
    vmax = int(min(j2 + j3 + m1, j3 - j1 + j2, j3 + m3))
    C = sqrt((2 * j3 + 1) * f(j3 + j1 - j2) * f(j3 - j1 + j2) * f(j1 + j2 - j3) / f(j1 + j2 + j3 + 1)
             * f(j3 + m3) * f(j3 - m3) / (f(j1 + m1) * f(j1 - m1) * f(j2 + m2) * f(j2 - m2)))
    S = 0.0
    for v in range(vmin, vmax + 1):
        S += (-1) ** (v + j2 + m2) * f(j2 + j3 + m1 - v) * f(j1 - m1 + v) / (
            f(v) * f(j3 - j1 + j2 - v) * f(j3 + m3 - v) * f(v + j1 - j2 - m3))
    return C * S


def _su2_clebsch_gordan(j1, j2, j3):
    C = np.zeros((2 * j1 + 1, 2 * j2 + 1, 2 * j3 + 1))
    for m1 in range(-j1, j1 + 1):
        for m2 in range(-j2, j2 + 1):
            m3 = m1 + m2
            if -j3 <= m3 <= j3:
                C[j1 + m1, j2 + m2, j3 + m3] = _su2_cg(j1, m1, j2, m2, j3, m3)
    return C


def _change_basis_real_to_complex(l):
    q = np.zeros((2 * l + 1, 2 * l + 1), dtype=np.complex128)
    for m in range(-l, 0):
        q[l + m, l + abs(m)] = 1.0 / sqrt(2)
        q[l + m, l - abs(m)] = -1j / sqrt(2)
    q[l, l] = 1.0
    for m in range(1, l + 1):
        q[l + m, l + abs(m)] = (-1) ** m / sqrt(2)
        q[l + m, l - abs(m)] = 1j * (-1) ** m / sqrt(2)
    return (-1j) ** l * q


def _so3_clebsch_gordan(l1, l2, l3):
    C = _su2_clebsch_gordan(l1, l2, l3).astype(np.complex128)
    Q1 = _change_basis_real_to_complex(l1)
    Q2 = _change_basis_real_to_complex(l2)
    Q3 = _change_basis_real_to_complex(l3)
    C = np.einsum('ij,kl,mn,ikn->jlm', Q1, Q2, np.conj(Q3.T), C)
    assert np.abs(C.imag).max() < 1e-10
    return C.real


def _build_combos():
    combos, cgs = [], []
    for ai, la in enumerate(INPUT_A_L):
        for hi, lh in enumerate(INPUT_H_L):
            if abs(la - lh) <= L_OUT <= la + lh:
                cg = _so3_clebsch_gordan(la, lh, L_OUT)
                if np.abs(cg).sum() > 0:
                    combos.append((ai, hi))
                    cgs.append(cg.astype(np.float32))
    return combos, cgs


VALID_COMBOS, CG_TENSORS = _build_combos()
CJ_OFF = []
NCJ = 0
for (_ai, _hi) in VALID_COMBOS:
    CJ_OFF.append(NCJ)
    NCJ += 2 * _hi + 1
NCOL = KO * NCJ  # 450


def _build_G():
    G = np.zeros((75, NCOL), dtype=np.float32)
    for c, (ai, hi) in enumerate(VALID_COMBOS):
        cg = CG_TENSORS[c]
        na, nh = 2 * ai + 1, 2 * hi + 1
        for k in range(KO):
            base = k * NCJ + CJ_OFF[c]
            G[ai * ai: ai * ai + na, base: base + nh] += cg[:, :, k]
            for j in range(nh):
                G[25 + hi * hi + j, base + j] = 1.0
                G[50 + c, base + j] = 1.0
    return G


G_CONST = _build_G()

# ---------------------------------------------------------------------------
# Bass kernel (raw, manual semaphores)
# ---------------------------------------------------------------------------
F32 = mybir.dt.float32
F16 = mybir.dt.float16
_NC = None


def build_nc():
    global _NC
    if _NC is not None:
        return _NC
    nc = bass.Bass(target_bir_lowering=False, enable_partition_id=False)

    # one packed input: cols 0-64 = block-diag A/H/w, cols 65.. = G (fp16)
    iv = nc.dram_tensor("iv", [75, 65 + NCOL], F16, kind="ExternalInput")
    out_dram = nc.dram_tensor("out", [KO], F32, kind="ExternalOutput")

    with ExitStack() as ctx:
        # one SBUF tensor: cols 0-64 = lhsT (block-diag inputs), 65.. = rhs G
        big = ctx.enter_context(nc.sbuf_tensor("big", [75, 65 + NCOL], F16))
        s1 = ctx.enter_context(nc.sbuf_tensor("s1", [1, NCJ], F32))
        hw = ctx.enter_context(nc.sbuf_tensor("hw", [1, NCJ], F32))
        prod = ctx.enter_context(nc.sbuf_tensor("prod", [1, NCOL], F32))
        o5 = ctx.enter_context(nc.sbuf_tensor("o5", [1, KO], F32))
        acc = ctx.enter_context(nc.psum_tensor("acc", [65, NCOL], F32))
        s_g = ctx.enter_context(nc.semaphore("s_g"))
        block = ctx.enter_context(nc.Block())

        @block.sync
        def _(sync):
            sync.dma_start(big[0:25, :], iv[0:25, :]).then_inc(s_g, 16)
            sync.dma_start(big[25:50, :], iv[25:50, :]).then_inc(s_g, 16)
            sync.wait_ge(s_g, 53)
            sync.dma_start(out_dram[:], o5[:]).then_inc(s_g, 16)

        @block.scalar
        def _(scalar):
            scalar.dma_start(big[50:75, :], iv[50:75, :]).then_inc(s_g, 16)

        @block.tensor
        def _(tensor):
            nc.tensor.matmul(acc[:], big[:, 0:65], big[:, 65:65 + NCOL],
                             start=True, stop=True) \
                .wait_op(s_g, 48, "sem-ge").then_inc(s_g, 1)

        @block.vector
        def _(vector):
            nc.vector.tensor_copy(s1[:], acc[32:33, 0:NCJ]) \
                .wait_op(s_g, 49, "sem-ge").then_inc(s_g, 1)
            nc.vector.tensor_mul(hw[:], s1[:], acc[64:65, 0:NCJ]) \
                .wait_op(s_g, 50, "sem-ge").then_inc(s_g, 1)
            hw_bcast = bass.AP(hw, 0, [[NCJ, 1], [0, KO], [1, NCJ]])
            nc.vector.tensor_mul(
                prod[:].rearrange("p (a b) -> p a b", b=NCJ),
                acc[0:1, :].rearrange("p (a b) -> p a b", b=NCJ),
                hw_bcast).wait_op(s_g, 51, "sem-ge").then_inc(s_g, 1)
            nc.vector.reduce_sum(
                o5[:], prod[:].rearrange("p (a b) -> p a b", b=NCJ),
                axis=mybir.AxisListType.X) \
                .wait_op(s_g, 52, "sem-ge").then_inc(s_g, 1)

    _NC = nc
    return nc


_G16PAD = np.zeros((75, NCOL), dtype=np.float16)
_G16PAD[:] = G_CONST.astype(np.float16)


def _pack_iv(inputs) -> np.ndarray:
    iv = np.zeros((75, 65 + NCOL), dtype=np.float16)
    iv[:, 65:] = _G16PAD
    for i, l in enumerate(INPUT_A_L):
        iv[l * l:(l + 1) * (l + 1), 0] = np.asarray(inputs[f"a{i}"], np.float16)
    for i, l in enumerate(INPUT_H_L):
        iv[25 + l * l:25 + (l + 1) * (l + 1), 32] = np.asarray(inputs[f"h{i}"], np.float16)
    iv[50:75, 64] = np.asarray(inputs["weight"], np.float16)
    return iv


def kernel(**inputs) -> np.ndarray:
    nc = build_nc()
    in_map = {"iv": _pack_iv(inputs)}
    core_ids = list(range(8))
    res = run_bass_kernel_spmd(nc, [dict(in_map) for _ in core_ids], core_ids)
    return res.results[0]["out"]
